# revision 50
# baseline (speedup 1.0000x reference)
"""Multi-head attention (B=4, N=2048, C=768, H=12, D=64) on 8 TRN2 NeuronCores.

Sharding: core c handles batch c//2 and query rows (c%2)*1024 .. +1024, all
heads. Each core recomputes K/V for its full batch (cheaper than any
collective), so there is no cross-core communication at all. The host ROTATES
x[b].T per core so that this core's query rows are always columns 0:1024 —
attention is permutation-invariant over keys, so k/v built from the rotated
sequence give identical results and the SPMD graph stays core-independent.

Layouts (host pre-transposes; contraction dim always on SBUF partitions):
  qT/kT = (wqkvT.T @ xT-slices), v natural = xT-tile.T @ wvT, packed per
  key-tile as [v_h | ones] so the AV matmul also produces the softmax
  denominator in PSUM row 64. scoresT[keys, q] = kT_tile.T @ qT; exp on
  ScalarE over [128, 1024] PSUM tiles (two N=512 matmuls fill one tile; the
  wide activation amortizes ACT's ~352-cycle per-instruction overhead); no max
  subtraction (scores are O(1) by construction). AV matmuls for head h-1
  interleave with scores/exp of head h: their inputs are all ready, so the PE
  stream has no ACT-dependent stalls. Normalization: reciprocal_approx_fast on
  the sums row, gpsimd partition-broadcast, fused into the PSUM->SBUF evict.
The 1/sqrt(D) scale is folded into the q columns of wqkvT on the host.
"""

from contextlib import ExitStack

import ml_dtypes
import numpy as np

import concourse.bass as bass
import concourse.tile as tile
from concourse import bacc, mybir
from concourse import bass_utils

B, N, C, H, Dh = 4, 2048, 768, 12, 64
P = 128
NCORES = 8
ROWS = N // 2  # query rows per core
SCALE = Dh ** -0.5

BF16 = mybir.dt.bfloat16
F32 = mybir.dt.float32

CB = C // P       # 6 contraction bands
NT = N // P       # 16 key tiles
QC = ROWS // 512  # 2 query half-chunks (N=512 matmuls)
KCH = N // 1024   # 2 key eviction chunks for kT

_cached_nc = None
LAST_RESULT = None  # BassKernelResults of the most recent run (for test harness)


def _build_nc():
    nc = bacc.Bacc(
        "TRN2",
        target_bir_lowering=False,
        debug=False,
        enable_asserts=False,
        num_devices=NCORES,
    )
    xT_d = nc.dram_tensor("xT", [C, N], BF16, kind="ExternalInput")
    wqkvT_d = nc.dram_tensor("wqkvT", [C, 3 * C], BF16, kind="ExternalInput")
    wprojT_d = nc.dram_tensor("wprojT", [C, C], BF16, kind="ExternalInput")
    bproj_d = nc.dram_tensor("bproj", [CB, P, 1], F32, kind="ExternalInput")
    out_d = nc.dram_tensor("out", [C, ROWS], F32, kind="ExternalOutput")

    Exp = mybir.ActivationFunctionType.Exp

    with tile.TileContext(nc) as tc:
        with ExitStack() as ctx:
            # ---- persistent pools ----
            pool_wp = ctx.enter_context(tc.tile_pool(name="wproj", bufs=1))
            pool_bias = ctx.enter_context(tc.tile_pool(name="bias", bufs=1))
            pool_qT = ctx.enter_context(tc.tile_pool(name="qT", bufs=1))
            pool_kT = ctx.enter_context(tc.tile_pool(name="kT", bufs=1))
            pool_vo = ctx.enter_context(tc.tile_pool(name="vones", bufs=1))
            pool_attT = ctx.enter_context(tc.tile_pool(name="attT", bufs=1))

            wp_sb = [pool_wp.tile([P, C], BF16, name=f"wp{i}") for i in range(CB)]
            bias_sb = [pool_bias.tile([P, 1], F32, name=f"bias{i}") for i in range(CB)]
            qT_sb = [pool_qT.tile([P, ROWS], BF16, name=f"qT{i}") for i in range(CB)]
            kT_sb = [pool_kT.tile([P, N], BF16, name=f"kT{i}") for i in range(CB)]
            # per key-tile: 12 heads x [v_h (64 cols) | ones (1 col)]
            vo_sb = [pool_vo.tile([P, H * (Dh + 1)], BF16, name=f"vo{i}") for i in range(NT)]
            attT_sb = [pool_attT.tile([P, ROWS], BF16, name=f"attT{i}") for i in range(CB)]

            for nt in range(NT):
                nc.gpsimd.memset(vo_sb[nt][:], 1.0)

            # ---- stage 1: qkv projections (own scope; pools freed after) ----
            with ExitStack() as s1:
                pool_x = s1.enter_context(tc.tile_pool(name="xT", bufs=1))
                pool_wqkv = s1.enter_context(tc.tile_pool(name="wqkv", bufs=1))
                ps_qk = s1.enter_context(tc.tile_pool(name="ps_qk", bufs=2, space="PSUM"))
                ps_v = s1.enter_context(tc.tile_pool(name="ps_v", bufs=2, space="PSUM"))

                x_sb = [pool_x.tile([P, N], BF16, name=f"x{i}") for i in range(CB)]
                wqkv_sb = [pool_wqkv.tile([P, 3 * C], BF16, name=f"wqkv{i}") for i in range(CB)]
                # priority order: q-columns of x + q-section of wqkv first so the
                # first projection chains start ~7us earlier; bulk follows.
                for cb in range(CB):
                    nc.sync.dma_start(x_sb[cb][:, 0:ROWS], xT_d.ap()[cb * P:(cb + 1) * P, 0:ROWS])
                    nc.sync.dma_start(wqkv_sb[cb][:, 0:2 * P], wqkvT_d.ap()[cb * P:(cb + 1) * P, 0:2 * P])
                for cb in range(CB):
                    nc.sync.dma_start(wqkv_sb[cb][:, 2 * P:C], wqkvT_d.ap()[cb * P:(cb + 1) * P, 2 * P:C])
                for cb in range(CB):
                    nc.sync.dma_start(x_sb[cb][:, ROWS:N], xT_d.ap()[cb * P:(cb + 1) * P, ROWS:N])
                    nc.sync.dma_start(wqkv_sb[cb][:, C:3 * C], wqkvT_d.ap()[cb * P:(cb + 1) * P, C:3 * C])
                for cb in range(CB):
                    nc.sync.dma_start(wp_sb[cb][:], wprojT_d.ap()[cb * P:(cb + 1) * P, :])
                    nc.sync.dma_start(bias_sb[cb][:], bproj_d.ap()[cb, :, :])

                # qT[o, n]: this core's query rows = x columns 0:1024 (host-rotated)
                for ob in range(CB):
                    pt = ps_qk.tile([P, 1024], F32, name="pt_q", tag="pt_qk")
                    for cb in range(CB):  # cb outer: both qc halves share one stationary
                        for qc in range(QC):
                            nc.tensor.matmul(
                                pt[:, qc * 512:(qc + 1) * 512],
                                wqkv_sb[cb][:, ob * P:(ob + 1) * P],
                                x_sb[cb][:, qc * 512:(qc + 1) * 512],
                                start=(cb == 0),
                                stop=(cb == CB - 1),
                            )
                    nc.vector.tensor_copy(qT_sb[ob][:], pt[:])
                # kT[o, n]: kc=0 chains first (their x columns arrive earlier)
                for kc in range(KCH):
                    for ob in range(CB):
                        pt = ps_qk.tile([P, 1024], F32, name="pt_k", tag="pt_qk")
                        for cb in range(CB):  # cb outer: halves share one stationary
                            for half in range(2):
                                nc.tensor.matmul(
                                    pt[:, half * 512:(half + 1) * 512],
                                    wqkv_sb[cb][:, C + ob * P:C + (ob + 1) * P],
                                    x_sb[cb][:, kc * 1024 + half * 512:kc * 1024 + (half + 1) * 512],
                                    start=(cb == 0),
                                    stop=(cb == CB - 1),
                                )
                        nc.vector.tensor_copy(kT_sb[ob][:, kc * 1024:(kc + 1) * 1024], pt[:])
                # v natural [n, c]; evict all 12 heads at once via 3D AP into [v|1] tiles
                for nt in range(NT):
                    pt = ps_v.tile([P, C], F32, name="pt_v")
                    for cb in range(CB):  # cb outer: x-tile stationary shared by chunks
                        for off, width in ((0, 512), (512, 256)):  # bank-aligned
                            nc.tensor.matmul(
                                pt[:, off:off + width],
                                x_sb[cb][:, nt * P:(nt + 1) * P],
                                wqkv_sb[cb][:, 2 * C + off:2 * C + off + width],
                                start=(cb == 0),
                                stop=(cb == CB - 1),
                            )
                    nc.vector.tensor_copy(
                        vo_sb[nt].rearrange("p (h e) -> p h e", e=Dh + 1)[:, :, 0:Dh],
                        pt[:].rearrange("p (h e) -> p h e", e=Dh),
                    )

            # ---- stage 2: attention; AV of head h-1 rides behind scores/exp of h ----
            pool_u = ctx.enter_context(tc.tile_pool(name="u", bufs=40))
            pool_r = ctx.enter_context(tc.tile_pool(name="r", bufs=4))
            pool_rb = ctx.enter_context(tc.tile_pool(name="rb", bufs=4))
            pool_y = ctx.enter_context(tc.tile_pool(name="y", bufs=3))
            ps_s = ctx.enter_context(tc.tile_pool(name="ps_s", bufs=2, space="PSUM"))
            ps_u = ctx.enter_context(tc.tile_pool(name="ps_u", bufs=2, space="PSUM"))

            uts = {}   # (h, kt) -> uT tile
            pus = {}   # h -> pu accumulator tile

            def emit_scores(h):
                band, hp = divmod(h, 2)
                po = hp * 64
                for kt in range(NT):
                    ps = ps_s.tile([P, 1024], F32, name="ps")
                    for qc in range(QC):
                        nc.tensor.matmul(
                            ps[:, qc * 512:(qc + 1) * 512],
                            kT_sb[band][po:po + 64, kt * P:(kt + 1) * P],
                            qT_sb[band][po:po + 64, qc * 512:(qc + 1) * 512],
                            start=True,
                            stop=True,
                        )
                    ut = pool_u.tile([P, 1024], BF16, name="ut")
                    nc.scalar.activation(ut[:], ps[:], Exp)
                    uts[(h, kt)] = ut
                    yield

            def emit_av(h):
                pu = ps_u.tile([P, 1024], F32, name="pu")
                pus[h] = pu
                for kt in range(NT):
                    for qc in range(QC):
                        nc.tensor.matmul(
                            pu[0:65, qc * 512:(qc + 1) * 512],
                            vo_sb[kt][:, h * 65:(h + 1) * 65],
                            uts[(h, kt)][:, qc * 512:(qc + 1) * 512],
                            start=(kt == 0),
                            stop=(kt == NT - 1),
                        )
                    yield

            def emit_normalize(h, split=False):
                band, hp = divmod(h, 2)
                po = hp * 64
                pu = pus.pop(h)
                s = pool_r.tile([1, ROWS], F32, name="s", tag="r")
                nc.vector.tensor_copy(s[:], pu[64:65, :])
                r = pool_r.tile([1, ROWS], F32, name="r", tag="r")
                nc.vector.reciprocal_approx_fast(r[:], s[:])
                rb = pool_rb.tile([64, ROWS], F32, name="rb")
                if split:  # halves pipelined so proj's first chunk unblocks sooner
                    for qc in range(QC):
                        sl = slice(qc * 512, (qc + 1) * 512)
                        nc.gpsimd.partition_broadcast(rb[:, sl], r[:, sl])
                        nc.vector.tensor_mul(
                            attT_sb[band][po:po + 64, sl], pu[0:64, sl], rb[:, sl]
                        )
                else:
                    nc.gpsimd.partition_broadcast(rb[:], r[:])
                    nc.vector.tensor_mul(attT_sb[band][po:po + 64, :], pu[0:64, :], rb[:])
                for kt in range(NT):
                    del uts[(h, kt)]

            for h in range(H):
                sc = emit_scores(h)
                av = emit_av(h - 1) if h > 0 else None
                for kt in range(0, NT, 2):  # kt-pair granularity halves PSUM
                    next(sc)                # bank-group switches on the PE
                    next(sc)
                    if av is not None:
                        next(av, None)
                        next(av, None)
                if av is not None:
                    emit_normalize(h - 1)
            for _ in emit_av(H - 1):
                pass
            emit_normalize(H - 1, split=True)

            # ---- stage 3: output projection (psum shared with ps_u slots) ----
            for ob in range(CB):
                for qc in range(QC):
                    pt = ps_u.tile([P, 512], F32, name="pt_y", tag="pu")
                    for cb in range(CB):
                        nc.tensor.matmul(
                            pt[:],
                            wp_sb[cb][:, ob * P:(ob + 1) * P],
                            attT_sb[cb][:, qc * 512:(qc + 1) * 512],
                            start=(cb == 0),
                            stop=(cb == CB - 1),
                        )
                    y = pool_y.tile([P, 512], F32, name="y")
                    nc.vector.tensor_scalar_add(y[:], pt[:], bias_sb[ob][:])
                    nc.sync.dma_start(
                        out_d.ap()[ob * P:(ob + 1) * P, qc * 512:(qc + 1) * 512], y[:]
                    )

    nc.compile()
    return nc


def kernel(x, w_qkv, w_proj, b_proj):
    global _cached_nc, LAST_RESULT
    if _cached_nc is None:
        _cached_nc = _build_nc()
    nc = _cached_nc

    x = np.asarray(x, dtype=np.float32)
    w_qkv = np.asarray(w_qkv, dtype=np.float32)
    w_proj = np.asarray(w_proj, dtype=np.float32)
    b_proj = np.asarray(b_proj, dtype=np.float32)

    bf = ml_dtypes.bfloat16
    wqkvT = w_qkv.T.astype(np.float32).copy()  # [C, 3C]
    wqkvT[:, :C] *= SCALE  # fold q scaling
    wqkvT = np.ascontiguousarray(wqkvT).astype(bf)
    wprojT = np.ascontiguousarray(w_proj.T).astype(bf)
    bproj_dev = np.ascontiguousarray(b_proj.astype(np.float32).reshape(CB, P, 1))

    in_maps = []
    for c in range(NCORES):
        b, half = divmod(c, 2)
        xTb = x[b].T.astype(bf)  # [C, N]
        if half:
            xTb = np.roll(xTb, -ROWS, axis=1)  # query rows -> columns 0:1024
        in_maps.append(
            {
                "xT": np.ascontiguousarray(xTb),
                "wqkvT": wqkvT,
                "wprojT": wprojT,
                "bproj": bproj_dev,
            }
        )

    res = bass_utils.run_bass_kernel_spmd(nc, in_maps, core_ids=list(range(NCORES)))
    LAST_RESULT = res

    out = np.empty((B, N, C), np.float32)
    for c in range(NCORES):
        b, half = divmod(c, 2)
        out[b, half * ROWS:(half + 1) * ROWS, :] = res.results[c]["out"].T
    return out


# revision 51
# speedup vs baseline: 1.0193x; 1.0193x over previous
"""Multi-head attention (B=4, N=2048, C=768, H=12, D=64) on 8 TRN2 NeuronCores.

Sharding: core c handles batch c//2 and query rows (c%2)*1024 .. +1024, all
heads. Each core recomputes K/V for its full batch (cheaper than any
collective), so there is no cross-core communication at all. The host ROTATES
x[b].T per core so that this core's query rows are always columns 0:1024 —
attention is permutation-invariant over keys, so k/v built from the rotated
sequence give identical results and the SPMD graph stays core-independent.

Layouts (host pre-transposes; contraction dim always on SBUF partitions):
  qT/kT = (wqkvT.T @ xT-slices), v natural = xT-tile.T @ wvT, packed per
  key-tile as [v_h | ones] so the AV matmul also produces the softmax
  denominator in PSUM row 64. scoresT[keys, q] = kT_tile.T @ qT; exp on
  ScalarE over [128, 1024] PSUM tiles (two N=512 matmuls fill one tile; the
  wide activation amortizes ACT's ~352-cycle per-instruction overhead); no max
  subtraction (scores are O(1) by construction). AV matmuls for head h-1
  interleave with scores/exp of head h: their inputs are all ready, so the PE
  stream has no ACT-dependent stalls. Normalization: reciprocal_approx_fast on
  the sums row, gpsimd partition-broadcast, fused into the PSUM->SBUF evict.
The 1/sqrt(D) scale is folded into the q columns of wqkvT on the host.
"""

from contextlib import ExitStack

import ml_dtypes
import numpy as np

import concourse.bass as bass
import concourse.tile as tile
from concourse import bacc, mybir
from concourse import bass_utils

B, N, C, H, Dh = 4, 2048, 768, 12, 64
P = 128
NCORES = 8
ROWS = N // 2  # query rows per core
SCALE = Dh ** -0.5

BF16 = mybir.dt.bfloat16
F32 = mybir.dt.float32

CB = C // P       # 6 contraction bands
NT = N // P       # 16 key tiles
QC = ROWS // 512  # 2 query half-chunks (N=512 matmuls)
KCH = N // 1024   # 2 key eviction chunks for kT

_cached_nc = None
LAST_RESULT = None  # BassKernelResults of the most recent run (for test harness)


def _build_nc():
    nc = bacc.Bacc(
        "TRN2",
        target_bir_lowering=False,
        debug=False,
        enable_asserts=False,
        num_devices=NCORES,
    )
    xT_d = nc.dram_tensor("xT", [C, N], BF16, kind="ExternalInput")
    wqkvT_d = nc.dram_tensor("wqkvT", [C, 3 * C], BF16, kind="ExternalInput")
    wprojT_d = nc.dram_tensor("wprojT", [C, C], BF16, kind="ExternalInput")
    bproj_d = nc.dram_tensor("bproj", [CB, P, 1], F32, kind="ExternalInput")
    out_d = nc.dram_tensor("out", [C, ROWS], F32, kind="ExternalOutput")

    Exp = mybir.ActivationFunctionType.Exp

    with tile.TileContext(nc) as tc:
        with ExitStack() as ctx:
            # ---- persistent pools ----
            pool_wp = ctx.enter_context(tc.tile_pool(name="wproj", bufs=1))
            pool_bias = ctx.enter_context(tc.tile_pool(name="bias", bufs=1))
            pool_qT = ctx.enter_context(tc.tile_pool(name="qT", bufs=1))
            pool_kT = ctx.enter_context(tc.tile_pool(name="kT", bufs=1))
            pool_vo = ctx.enter_context(tc.tile_pool(name="vones", bufs=1))
            pool_attT = ctx.enter_context(tc.tile_pool(name="attT", bufs=1))

            wp_sb = [pool_wp.tile([P, C], BF16, name=f"wp{i}") for i in range(CB)]
            bias_sb = [pool_bias.tile([P, 1], F32, name=f"bias{i}") for i in range(CB)]
            qT_sb = [pool_qT.tile([P, ROWS], BF16, name=f"qT{i}") for i in range(CB)]
            kT_sb = [pool_kT.tile([P, N], BF16, name=f"kT{i}") for i in range(CB)]
            # per key-tile: 12 heads x [v_h (64 cols) | ones (1 col)]
            vo_sb = [pool_vo.tile([P, H * (Dh + 1)], BF16, name=f"vo{i}") for i in range(NT)]
            attT_sb = [pool_attT.tile([P, ROWS], BF16, name=f"attT{i}") for i in range(CB)]

            for nt in range(NT):
                nc.gpsimd.memset(vo_sb[nt][:], 1.0)

            # ---- stage 1: qkv projections (own scope; pools freed after) ----
            with ExitStack() as s1:
                pool_x = s1.enter_context(tc.tile_pool(name="xT", bufs=1))
                pool_wqkv = s1.enter_context(tc.tile_pool(name="wqkv", bufs=1))
                ps_qk = s1.enter_context(tc.tile_pool(name="ps_qk", bufs=2, space="PSUM"))
                ps_v = s1.enter_context(tc.tile_pool(name="ps_v", bufs=2, space="PSUM"))

                x_sb = [pool_x.tile([P, N], BF16, name=f"x{i}") for i in range(CB)]
                wqkv_sb = [pool_wqkv.tile([P, 3 * C], BF16, name=f"wqkv{i}") for i in range(CB)]
                # priority order: q-columns of x + q-section of wqkv first so the
                # first projection chains start ~7us earlier; bulk follows.
                for cb in range(CB):
                    nc.sync.dma_start(x_sb[cb][:, 0:ROWS], xT_d.ap()[cb * P:(cb + 1) * P, 0:ROWS])
                    nc.sync.dma_start(wqkv_sb[cb][:, 0:2 * P], wqkvT_d.ap()[cb * P:(cb + 1) * P, 0:2 * P])
                for cb in range(CB):
                    nc.sync.dma_start(wqkv_sb[cb][:, 2 * P:C], wqkvT_d.ap()[cb * P:(cb + 1) * P, 2 * P:C])
                for cb in range(CB):
                    nc.sync.dma_start(x_sb[cb][:, ROWS:N], xT_d.ap()[cb * P:(cb + 1) * P, ROWS:N])
                    nc.sync.dma_start(wqkv_sb[cb][:, C:3 * C], wqkvT_d.ap()[cb * P:(cb + 1) * P, C:3 * C])
                for cb in range(CB):
                    nc.sync.dma_start(wp_sb[cb][:], wprojT_d.ap()[cb * P:(cb + 1) * P, :])
                    nc.sync.dma_start(bias_sb[cb][:], bproj_d.ap()[cb, :, :])

                # qT[o, n]: this core's query rows = x columns 0:1024 (host-rotated)
                for ob in range(CB):
                    pt = ps_qk.tile([P, 1024], F32, name="pt_q", tag="pt_qk")
                    for cb in range(CB):  # cb outer: both qc halves share one stationary
                        for qc in range(QC):
                            nc.tensor.matmul(
                                pt[:, qc * 512:(qc + 1) * 512],
                                wqkv_sb[cb][:, ob * P:(ob + 1) * P],
                                x_sb[cb][:, qc * 512:(qc + 1) * 512],
                                start=(cb == 0),
                                stop=(cb == CB - 1),
                            )
                    nc.vector.tensor_copy(qT_sb[ob][:], pt[:])
                # kT[o, n]: kc=0 chains first (their x columns arrive earlier)
                for kc in range(KCH):
                    for ob in range(CB):
                        pt = ps_qk.tile([P, 1024], F32, name="pt_k", tag="pt_qk")
                        for cb in range(CB):  # cb outer: halves share one stationary
                            for half in range(2):
                                nc.tensor.matmul(
                                    pt[:, half * 512:(half + 1) * 512],
                                    wqkv_sb[cb][:, C + ob * P:C + (ob + 1) * P],
                                    x_sb[cb][:, kc * 1024 + half * 512:kc * 1024 + (half + 1) * 512],
                                    start=(cb == 0),
                                    stop=(cb == CB - 1),
                                )
                        nc.vector.tensor_copy(kT_sb[ob][:, kc * 1024:(kc + 1) * 1024], pt[:])
                # v natural [n, c]; evict all 12 heads at once via 3D AP into [v|1] tiles
                for nt in range(NT):
                    pt = ps_v.tile([P, C], F32, name="pt_v")
                    for cb in range(CB):  # cb outer: x-tile stationary shared by chunks
                        for off, width in ((0, 512), (512, 256)):  # bank-aligned
                            nc.tensor.matmul(
                                pt[:, off:off + width],
                                x_sb[cb][:, nt * P:(nt + 1) * P],
                                wqkv_sb[cb][:, 2 * C + off:2 * C + off + width],
                                start=(cb == 0),
                                stop=(cb == CB - 1),
                            )
                    nc.vector.tensor_copy(
                        vo_sb[nt].rearrange("p (h e) -> p h e", e=Dh + 1)[:, :, 0:Dh],
                        pt[:].rearrange("p (h e) -> p h e", e=Dh),
                    )

            # ---- stage 2: attention; AV of head h-1 rides behind scores/exp of h ----
            pool_u = ctx.enter_context(tc.tile_pool(name="u", bufs=40))
            pool_r = ctx.enter_context(tc.tile_pool(name="r", bufs=4))
            pool_rb = ctx.enter_context(tc.tile_pool(name="rb", bufs=4))
            pool_y = ctx.enter_context(tc.tile_pool(name="y", bufs=3))
            ps_s = ctx.enter_context(tc.tile_pool(name="ps_s", bufs=2, space="PSUM"))
            ps_u = ctx.enter_context(tc.tile_pool(name="ps_u", bufs=2, space="PSUM"))

            uts = {}   # (h, kt) -> uT tile
            pus = {}   # h -> pu accumulator tile

            def emit_scores(h):
                band, hp = divmod(h, 2)
                po = hp * 64
                for kt in range(NT):
                    ps = ps_s.tile([P, 1024], F32, name="ps")
                    for qc in range(QC):
                        nc.tensor.matmul(
                            ps[:, qc * 512:(qc + 1) * 512],
                            kT_sb[band][po:po + 64, kt * P:(kt + 1) * P],
                            qT_sb[band][po:po + 64, qc * 512:(qc + 1) * 512],
                            start=True,
                            stop=True,
                        )
                    ut = pool_u.tile([P, 1024], BF16, name="ut")
                    nc.scalar.activation(ut[:], ps[:], Exp)
                    uts[(h, kt)] = ut
                    yield

            def emit_av(h):
                pu = ps_u.tile([P, 1024], F32, name="pu")
                pus[h] = pu
                for kt in range(NT):
                    for qc in range(QC):
                        nc.tensor.matmul(
                            pu[0:65, qc * 512:(qc + 1) * 512],
                            vo_sb[kt][:, h * 65:(h + 1) * 65],
                            uts[(h, kt)][:, qc * 512:(qc + 1) * 512],
                            start=(kt == 0),
                            stop=(kt == NT - 1),
                        )
                    yield

            def emit_normalize(h, split=False):
                band, hp = divmod(h, 2)
                po = hp * 64
                pu = pus.pop(h)
                s = pool_r.tile([1, ROWS], F32, name="s", tag="r")
                nc.vector.tensor_copy(s[:], pu[64:65, :])
                r = pool_r.tile([1, ROWS], F32, name="r", tag="r")
                nc.vector.reciprocal_approx_fast(r[:], s[:])
                rb = pool_rb.tile([64, ROWS], F32, name="rb")
                if split:  # halves pipelined so proj's first chunk unblocks sooner
                    for qc in range(QC):
                        sl = slice(qc * 512, (qc + 1) * 512)
                        nc.gpsimd.partition_broadcast(rb[:, sl], r[:, sl])
                        nc.vector.tensor_mul(
                            attT_sb[band][po:po + 64, sl], pu[0:64, sl], rb[:, sl]
                        )
                else:
                    nc.gpsimd.partition_broadcast(rb[:], r[:])
                    nc.vector.tensor_mul(attT_sb[band][po:po + 64, :], pu[0:64, :], rb[:])
                for kt in range(NT):
                    del uts[(h, kt)]

            for h in range(H):
                sc = emit_scores(h)
                av = emit_av(h - 1) if h > 0 else None
                for kt in range(0, NT, 2):  # kt-pair granularity halves PSUM
                    next(sc)                # bank-group switches on the PE
                    next(sc)
                    if av is not None:
                        next(av, None)
                        next(av, None)
                if av is not None:
                    emit_normalize(h - 1)
            for _ in emit_av(H - 1):
                pass
            emit_normalize(H - 1, split=True)

            # ---- stage 3: output projection (psum shared with ps_u slots) ----
            for ob in range(CB):
                for qc in range(QC):
                    pt = ps_s.tile([P, 512], F32, name="pt_y", tag="ps")
                    for cb in range(CB):
                        nc.tensor.matmul(
                            pt[:],
                            wp_sb[cb][:, ob * P:(ob + 1) * P],
                            attT_sb[cb][:, qc * 512:(qc + 1) * 512],
                            start=(cb == 0),
                            stop=(cb == CB - 1),
                        )
                    y = pool_y.tile([P, 512], F32, name="y")
                    nc.vector.tensor_scalar_add(y[:], pt[:], bias_sb[ob][:])
                    nc.sync.dma_start(
                        out_d.ap()[ob * P:(ob + 1) * P, qc * 512:(qc + 1) * 512], y[:]
                    )

    nc.compile()
    return nc


def kernel(x, w_qkv, w_proj, b_proj):
    global _cached_nc, LAST_RESULT
    if _cached_nc is None:
        _cached_nc = _build_nc()
    nc = _cached_nc

    x = np.asarray(x, dtype=np.float32)
    w_qkv = np.asarray(w_qkv, dtype=np.float32)
    w_proj = np.asarray(w_proj, dtype=np.float32)
    b_proj = np.asarray(b_proj, dtype=np.float32)

    bf = ml_dtypes.bfloat16
    wqkvT = w_qkv.T.astype(np.float32).copy()  # [C, 3C]
    wqkvT[:, :C] *= SCALE  # fold q scaling
    wqkvT = np.ascontiguousarray(wqkvT).astype(bf)
    wprojT = np.ascontiguousarray(w_proj.T).astype(bf)
    bproj_dev = np.ascontiguousarray(b_proj.astype(np.float32).reshape(CB, P, 1))

    in_maps = []
    for c in range(NCORES):
        b, half = divmod(c, 2)
        xTb = x[b].T.astype(bf)  # [C, N]
        if half:
            xTb = np.roll(xTb, -ROWS, axis=1)  # query rows -> columns 0:1024
        in_maps.append(
            {
                "xT": np.ascontiguousarray(xTb),
                "wqkvT": wqkvT,
                "wprojT": wprojT,
                "bproj": bproj_dev,
            }
        )

    res = bass_utils.run_bass_kernel_spmd(nc, in_maps, core_ids=list(range(NCORES)))
    LAST_RESULT = res

    out = np.empty((B, N, C), np.float32)
    for c in range(NCORES):
        b, half = divmod(c, 2)
        out[b, half * ROWS:(half + 1) * ROWS, :] = res.results[c]["out"].T
    return out
